# revision 3
# baseline (speedup 1.0000x reference)
"""PointPillarsScatter Trainium2 Bass kernel (8-core SPMD, data parallel).

Problem: scatter M=100000 pillar feature rows (C=64, fp32) into a
(B=4, C=64, NY=512, NX=512) canvas addressed by (batch, y, x)
coordinates. Duplicate coordinates resolve last-write-wins (matching
XLA CPU scatter .set; the neuron-backend reference is nondeterministic
under collisions, run-to-run noise ~1e-2 relative).

Sharding (data-parallel, no cross-core communication): core k owns
batch b = k//2 and y-half yh = k%2 — a (64, 256, 512) output slice =
131072 cells. Cells are processed as 65536 PAIRS (cell c, cell
c+65536), i.e. output rows y and y+128, so each gathered/scattered
element is 512B (full DMA descriptor rate) and each PE transpose
handles a full [128, 128] block.

Per-core device pipeline (16 regions x 4096 pair-slots):
- DVE/Pool memset two canvas tiles own/peer [128, 16, 128] f32 (a
  pair-slot s maps to partition s%128, free group s//256, tile chosen
  by parity (s//128)&1 — the hardware layout of SBUF-dst
  dma_scatter_add with sbuf_tokens_per_rank=128).
- One DMA loads the region's host-packed non-empty pair rows (wrapped
  [128, 6, 128] layout), one dma_scatter_add (CCE, fp32 add onto the
  zeroed tiles == placement) scatters them to their slots. Only
  ~700 of 4096 slots are non-empty, so this moves ~6x less data than
  gathering every cell. Padding descriptors are pointed at EMPTY
  slots: concurrent CCE read-modify-writes racing on one occupied
  address can drop a real pillar's add.
- 32 PE transpose-mode matmuls ([128 pairs, 128] -> [128, 128]; out
  partitions = channels of the A half (0:64) and B half (64:128))
  produce channel-major data in PSUM.
- scalar/vector engines copy PSUM -> SBUF, two DMAs per half-region
  write output rows y..y+3 and y+128..y+131 ([64 partitions, 8KB
  contiguous per channel] descriptors).

SWDGE note: one Pool-DMA instruction can carry at most ~1024
descriptors (default dynamic-DMA ring), hence num_idxs <= 1024.
"""

import sys

import numpy as np

_TRN_REPO = "/opt/trn_rl_repo"
if _TRN_REPO not in sys.path:
    sys.path.insert(0, _TRN_REPO)

NY, NX, C, B = 512, 512, 64, 4
CELLS = B * NY * NX             # 1048576
N_CORES = 8
CORE_CELLS = CELLS // N_CORES   # 131072
HALF = CORE_CELLS // 2          # 65536 pairs per core
REGIONS = 16
REGION_PAIRS = HALF // REGIONS  # 4096
MAX_NE = 768                    # scatter rows per region (observed max 740)
MAX_NE_FALLBACK = 1024          # recompile capacity if inputs ever differ
PAIR_ELEM = 2 * C               # 128 f32 = 512B


def build_nc(max_ne=MAX_NE):
    """Build the per-core Bass program (SPMD: same NEFF on all 8 cores)."""
    from concourse import bacc, masks, tile
    from concourse import mybir

    f32 = mybir.dt.float32
    i16 = mybir.dt.int16

    nc = bacc.Bacc(
        "TRN2", target_bir_lowering=False, debug=False, num_devices=N_CORES
    )
    table = nc.dram_tensor(
        "table", [REGIONS, max_ne, PAIR_ELEM], f32, kind="ExternalInput"
    )
    idx = nc.dram_tensor(
        "idx", [128, REGIONS * (max_ne // 16)], i16, kind="ExternalInput"
    )
    out = nc.dram_tensor("out", [C, CORE_CELLS], f32, kind="ExternalOutput")

    ncols = max_ne // 16

    with tile.TileContext(nc) as tc:
        with (
            tc.tile_pool(name="const", bufs=1) as cpool,
            tc.tile_pool(name="canvas", bufs=6) as canvas_pool,
            tc.tile_pool(name="srcp", bufs=6) as spool,
            tc.tile_pool(name="outp", bufs=6) as opool,
            tc.tile_pool(name="psum", bufs=2, space="PSUM") as ppool,
        ):
            ident = cpool.tile([128, 128], f32)
            masks.make_identity(nc, ident[:])
            idx_sb = cpool.tile([128, REGIONS * ncols], i16)
            nc.sync.dma_start(out=idx_sb[:], in_=idx[:])

            for g in range(REGIONS):
                own = canvas_pool.tile([128, REGIONS, PAIR_ELEM], f32, tag="own")
                peer = canvas_pool.tile([128, REGIONS, PAIR_ELEM], f32, tag="peer")
                nc.vector.memset(own[:], 0.0)
                nc.gpsimd.memset(peer[:], 0.0)

                src = spool.tile([128, max_ne // 128, PAIR_ELEM], f32)
                nc.sync.dma_start(
                    out=src[:], in_=table[g].rearrange("(c p) e -> p c e", p=128)
                )

                nc.gpsimd.dma_scatter_add(
                    out_ap=own[:],
                    in_ap=src[:],
                    idxs_ap=idx_sb[:, g * ncols:(g + 1) * ncols],
                    num_idxs=max_ne,
                    num_idxs_reg=max_ne,
                    elem_size=PAIR_ELEM,
                    parity_reg=0,
                    out_ap_other=peer[:],
                    sbuf_tokens_per_rank=128,
                )

                for h in range(2):
                    ps = ppool.tile([128, 2048], f32)
                    for jj in range(16):
                        j = 16 * h + jj
                        blk = (own if j % 2 == 0 else peer)[:, j // 2, :]
                        nc.tensor.transpose(
                            ps[:, 128 * jj:128 * (jj + 1)], blk, ident[:]
                        )
                    ot = opool.tile([128, 2048], f32)
                    if (2 * g + h) % 2 == 0:
                        nc.scalar.copy(ot[:], ps[:])
                    else:
                        nc.vector.tensor_copy(ot[:], ps[:])
                    start_a = g * REGION_PAIRS + h * 2048
                    nc.sync.dma_start(
                        out=out[0:C, start_a:start_a + 2048], in_=ot[0:C, :]
                    )
                    nc.sync.dma_start(
                        out=out[0:C, HALF + start_a:HALF + start_a + 2048],
                        in_=ot[C:2 * C, :],
                    )
    nc.compile()
    return nc


def host_prep(pillar_features, coordinates, max_ne):
    """Per-core {table, idx} maps. Last write wins on duplicate cells."""
    pf = np.ascontiguousarray(np.asarray(pillar_features), dtype=np.float32)
    coords = np.asarray(coordinates)
    m = pf.shape[0]
    flat = (
        coords[:, 0].astype(np.int64) * (NY * NX)
        + coords[:, 2].astype(np.int64) * NX
        + coords[:, 3].astype(np.int64)
    )
    order = np.argsort(flat, kind="stable")
    fs = flat[order]
    is_last = np.empty(m, dtype=bool)
    if m > 1:
        is_last[:-1] = fs[:-1] != fs[1:]
    is_last[-1] = True
    occ = np.full(CELLS, -1, dtype=np.int64)
    occ[fs[is_last]] = order[is_last]

    in_maps = []
    for k in range(N_CORES):
        b, yh = k // 2, k % 2
        base = b * (NY * NX) + yh * (NY // 2) * NX
        occ_k = occ[base: base + CORE_CELLS]
        p_a, p_b = occ_k[:HALF], occ_k[HALF:]

        tbl = np.zeros((REGIONS, max_ne, PAIR_ELEM), dtype=np.float32)
        idx_all = np.zeros((REGIONS, max_ne), dtype=np.int16)
        for g in range(REGIONS):
            sl = slice(g * REGION_PAIRS, (g + 1) * REGION_PAIRS)
            ra, rb = p_a[sl], p_b[sl]
            ne = np.where((ra >= 0) | (rb >= 0))[0]
            n = len(ne)
            if n > max_ne:
                return None  # caller retries with larger capacity
            m_a = ra[ne] >= 0
            m_b = rb[ne] >= 0
            tbl[g, :n][m_a, :C] = pf[ra[ne][m_a]]
            tbl[g, :n][m_b, C:] = pf[rb[ne][m_b]]
            idx_all[g, :n] = ne.astype(np.int16)
            # padding rows add zeros; target only EMPTY slots (a racing
            # CCE read-modify-write on an occupied slot can drop data)
            empty = np.setdiff1d(
                np.arange(REGION_PAIRS, dtype=np.int16),
                ne.astype(np.int16),
                assume_unique=True,
            )
            assert len(empty) > 0
            idx_all[g, n:] = np.resize(empty, max_ne - n)

        blk = idx_all.reshape(REGIONS, max_ne // 16, 16)
        blk = blk.transpose(2, 0, 1).reshape(16, REGIONS * (max_ne // 16))
        idx_tile = np.ascontiguousarray(np.tile(blk, (8, 1)))
        in_maps.append({"table": tbl, "idx": idx_tile})
    return in_maps


_NC_CACHE = {}


def _get_nc(max_ne):
    if max_ne not in _NC_CACHE:
        _NC_CACHE[max_ne] = build_nc(max_ne)
    return _NC_CACHE[max_ne]


def kernel(pillar_features, coordinates, batch_size):
    assert int(batch_size) == B
    from concourse.bass_utils import run_bass_kernel_spmd

    in_maps = host_prep(pillar_features, coordinates, MAX_NE)
    max_ne = MAX_NE
    if in_maps is None:
        max_ne = MAX_NE_FALLBACK
        in_maps = host_prep(pillar_features, coordinates, max_ne)
        assert in_maps is not None, "region occupancy exceeds fallback capacity"
    nc = _get_nc(max_ne)
    res = run_bass_kernel_spmd(nc, in_maps, list(range(N_CORES)))

    full = np.empty((B, C, NY, NX), dtype=np.float32)
    for k in range(N_CORES):
        b, yh = k // 2, k % 2
        out_k = res.results[k]["out"].reshape(C, NY // 2, NX)
        full[b, :, yh * (NY // 2):(yh + 1) * (NY // 2), :] = out_k
    return full


# revision 4
# speedup vs baseline: 1.0560x; 1.0560x over previous
"""PointPillarsScatter Trainium2 Bass kernel (8-core SPMD, data parallel).

Problem: scatter M=100000 pillar feature rows (C=64, fp32) into a
(B=4, C=64, NY=512, NX=512) canvas addressed by (batch, y, x)
coordinates. Duplicate coordinates resolve last-write-wins (matching
XLA CPU scatter .set; the neuron-backend reference is nondeterministic
under collisions, run-to-run noise ~1e-2 relative).

Sharding (data-parallel, no cross-core communication): core k owns
batch b = k//2 and y-half yh = k%2 — a (64, 256, 512) output slice =
131072 cells. Cells are processed as 65536 PAIRS (cell c, cell
c+65536), i.e. output rows y and y+128, so each gathered/scattered
element is 512B (full DMA descriptor rate) and each PE transpose
handles a full [128, 128] block.

Per-core device pipeline (16 regions x 4096 pair-slots):
- DVE/Pool memset two canvas tiles own/peer [128, 16, 128] f32 (a
  pair-slot s maps to partition s%128, free group s//256, tile chosen
  by parity (s//128)&1 — the hardware layout of SBUF-dst
  dma_scatter_add with sbuf_tokens_per_rank=128).
- One DMA loads the region's host-packed non-empty pair rows (wrapped
  [128, 6, 128] layout), one dma_scatter_add (CCE, fp32 add onto the
  zeroed tiles == placement) scatters them to their slots. Only
  ~700 of 4096 slots are non-empty, so this moves ~6x less data than
  gathering every cell. Padding descriptors are pointed at EMPTY
  slots: concurrent CCE read-modify-writes racing on one occupied
  address can drop a real pillar's add.
- 32 PE transpose-mode matmuls ([128 pairs, 128] -> [128, 128]; out
  partitions = channels of the A half (0:64) and B half (64:128))
  produce channel-major data in PSUM.
- scalar/vector engines copy PSUM -> SBUF, two DMAs per half-region
  write output rows y..y+3 and y+128..y+131 ([64 partitions, 8KB
  contiguous per channel] descriptors).

SWDGE note: one Pool-DMA instruction can carry at most ~1024
descriptors (default dynamic-DMA ring), hence num_idxs <= 1024.
"""

import sys

import numpy as np

_TRN_REPO = "/opt/trn_rl_repo"
if _TRN_REPO not in sys.path:
    sys.path.insert(0, _TRN_REPO)

NY, NX, C, B = 512, 512, 64, 4
CELLS = B * NY * NX             # 1048576
N_CORES = 8
CORE_CELLS = CELLS // N_CORES   # 131072
HALF = CORE_CELLS // 2          # 65536 pairs per core
REGIONS = 16
REGION_PAIRS = HALF // REGIONS  # 4096
MAX_NE = 768                    # scatter rows per region (observed max 740)
MAX_NE_FALLBACK = 1024          # recompile capacity if inputs ever differ
PAIR_ELEM = 2 * C               # 128 f32 = 512B


def build_nc(max_ne=MAX_NE):
    """Build the per-core Bass program (SPMD: same NEFF on all 8 cores)."""
    from concourse import bacc, masks, tile
    from concourse import mybir

    f32 = mybir.dt.float32
    i16 = mybir.dt.int16

    nc = bacc.Bacc(
        "TRN2", target_bir_lowering=False, debug=False, num_devices=N_CORES
    )
    table = nc.dram_tensor(
        "table", [REGIONS, max_ne, PAIR_ELEM], f32, kind="ExternalInput"
    )
    idx = nc.dram_tensor(
        "idx", [128, REGIONS * (max_ne // 16)], i16, kind="ExternalInput"
    )
    out = nc.dram_tensor("out", [C, CORE_CELLS], f32, kind="ExternalOutput")

    ncols = max_ne // 16

    with tile.TileContext(nc) as tc:
        with (
            tc.tile_pool(name="const", bufs=1) as cpool,
            tc.tile_pool(name="canvas", bufs=6) as canvas_pool,
            tc.tile_pool(name="srcp", bufs=6) as spool,
            tc.tile_pool(name="outp", bufs=6) as opool,
            tc.tile_pool(name="psum", bufs=2, space="PSUM") as ppool,
        ):
            ident = cpool.tile([128, 128], f32)
            masks.make_identity(nc, ident[:])
            idx_sb = cpool.tile([128, REGIONS * ncols], i16)
            nc.sync.dma_start(out=idx_sb[:], in_=idx[:])

            for g in range(REGIONS):
                own = canvas_pool.tile([128, REGIONS, PAIR_ELEM], f32, tag="own")
                peer = canvas_pool.tile([128, REGIONS, PAIR_ELEM], f32, tag="peer")
                nc.vector.memset(own[:], 0.0)
                nc.gpsimd.memset(peer[:], 0.0)

                src = spool.tile([128, max_ne // 128, PAIR_ELEM], f32)
                nc.sync.dma_start(
                    out=src[:], in_=table[g].rearrange("(c p) e -> p c e", p=128)
                )

                nc.gpsimd.dma_scatter_add(
                    out_ap=own[:],
                    in_ap=src[:],
                    idxs_ap=idx_sb[:, g * ncols:(g + 1) * ncols],
                    num_idxs=max_ne,
                    num_idxs_reg=max_ne,
                    elem_size=PAIR_ELEM,
                    parity_reg=0,
                    out_ap_other=peer[:],
                    sbuf_tokens_per_rank=128,
                )

                for h in range(2):
                    ps = ppool.tile([128, 2048], f32)
                    for jj in range(16):
                        j = 16 * h + jj
                        blk = (own if j % 2 == 0 else peer)[:, j // 2, :]
                        nc.tensor.transpose(
                            ps[:, 128 * jj:128 * (jj + 1)], blk, ident[:]
                        )
                    ot = opool.tile([128, 2048], f32)
                    if (2 * g + h) % 2 == 0:
                        nc.scalar.copy(ot[:], ps[:])
                    else:
                        nc.vector.tensor_copy(ot[:], ps[:])
                    # alternate output DMAs across the two HWDGE queues
                    # (SP / Activation) so descriptor generation pipelines
                    eng = nc.sync if h % 2 == 0 else nc.scalar
                    start_a = g * REGION_PAIRS + h * 2048
                    eng.dma_start(
                        out=out[0:C, start_a:start_a + 2048], in_=ot[0:C, :]
                    )
                    eng.dma_start(
                        out=out[0:C, HALF + start_a:HALF + start_a + 2048],
                        in_=ot[C:2 * C, :],
                    )
    nc.compile()
    return nc


def host_prep(pillar_features, coordinates, max_ne):
    """Per-core {table, idx} maps. Last write wins on duplicate cells."""
    pf = np.ascontiguousarray(np.asarray(pillar_features), dtype=np.float32)
    coords = np.asarray(coordinates)
    m = pf.shape[0]
    flat = (
        coords[:, 0].astype(np.int64) * (NY * NX)
        + coords[:, 2].astype(np.int64) * NX
        + coords[:, 3].astype(np.int64)
    )
    order = np.argsort(flat, kind="stable")
    fs = flat[order]
    is_last = np.empty(m, dtype=bool)
    if m > 1:
        is_last[:-1] = fs[:-1] != fs[1:]
    is_last[-1] = True
    occ = np.full(CELLS, -1, dtype=np.int64)
    occ[fs[is_last]] = order[is_last]

    in_maps = []
    for k in range(N_CORES):
        b, yh = k // 2, k % 2
        base = b * (NY * NX) + yh * (NY // 2) * NX
        occ_k = occ[base: base + CORE_CELLS]
        p_a, p_b = occ_k[:HALF], occ_k[HALF:]

        tbl = np.zeros((REGIONS, max_ne, PAIR_ELEM), dtype=np.float32)
        idx_all = np.zeros((REGIONS, max_ne), dtype=np.int16)
        for g in range(REGIONS):
            sl = slice(g * REGION_PAIRS, (g + 1) * REGION_PAIRS)
            ra, rb = p_a[sl], p_b[sl]
            ne = np.where((ra >= 0) | (rb >= 0))[0]
            n = len(ne)
            if n > max_ne:
                return None  # caller retries with larger capacity
            m_a = ra[ne] >= 0
            m_b = rb[ne] >= 0
            tbl[g, :n][m_a, :C] = pf[ra[ne][m_a]]
            tbl[g, :n][m_b, C:] = pf[rb[ne][m_b]]
            idx_all[g, :n] = ne.astype(np.int16)
            # padding rows add zeros; target only EMPTY slots (a racing
            # CCE read-modify-write on an occupied slot can drop data)
            empty = np.setdiff1d(
                np.arange(REGION_PAIRS, dtype=np.int16),
                ne.astype(np.int16),
                assume_unique=True,
            )
            assert len(empty) > 0
            idx_all[g, n:] = np.resize(empty, max_ne - n)

        blk = idx_all.reshape(REGIONS, max_ne // 16, 16)
        blk = blk.transpose(2, 0, 1).reshape(16, REGIONS * (max_ne // 16))
        idx_tile = np.ascontiguousarray(np.tile(blk, (8, 1)))
        in_maps.append({"table": tbl, "idx": idx_tile})
    return in_maps


_NC_CACHE = {}


def _get_nc(max_ne):
    if max_ne not in _NC_CACHE:
        _NC_CACHE[max_ne] = build_nc(max_ne)
    return _NC_CACHE[max_ne]


def kernel(pillar_features, coordinates, batch_size):
    assert int(batch_size) == B
    from concourse.bass_utils import run_bass_kernel_spmd

    in_maps = host_prep(pillar_features, coordinates, MAX_NE)
    max_ne = MAX_NE
    if in_maps is None:
        max_ne = MAX_NE_FALLBACK
        in_maps = host_prep(pillar_features, coordinates, max_ne)
        assert in_maps is not None, "region occupancy exceeds fallback capacity"
    nc = _get_nc(max_ne)
    res = run_bass_kernel_spmd(nc, in_maps, list(range(N_CORES)))

    full = np.empty((B, C, NY, NX), dtype=np.float32)
    for k in range(N_CORES):
        b, yh = k // 2, k % 2
        out_k = res.results[k]["out"].reshape(C, NY // 2, NX)
        full[b, :, yh * (NY // 2):(yh + 1) * (NY // 2), :] = out_k
    return full
